# revision 66
# baseline (speedup 1.0000x reference)
"""Multi-head self-attention (B=8, S=1024, D=1024, H=16) on 8 TRN2 NeuronCores.

Sharding: data-parallel over batch — one batch element per core, weights
replicated; no collectives needed.

Per-core kernel runs attention in a transposed layout so the only on-chip
transpose is X^T (64 PE transposes):
  X^T [d, s]            PE transpose of the input (f32r), stored bf16
  Q^T, K^T [c, s]       = W_{q,k}.T @ X^T   (channel tiles on partitions)
  V [s, c]              natural orientation, with a ones column per head
  scores^T [k, q]       = K_h @ Q_h^T       (contraction over head dim = 64)
  P^T = exp(scores^T)   no max subtraction (|scores| <~ 6 by construction)
  num^T [65, q]         = V'_h.T @ P^T      row 64 = softmax denominator
  attnout^T [c, q]      = num^T[0:64] * (1/denom)  (gpsimd partition_broadcast)
  out [s, d]            = attnout^T.T @ W_proj + b_proj
Key HW findings this kernel encodes (differential-timed on TRN2):
  - Every matmul operand except the X transposes is bf16: 2-byte stationary
    weight loads into the PE roughly halve per-matmul overhead (~80us/rep
    measured vs the all-f32r pipeline). QKV/proj weights are cast
    fp32->bf16 in-flight by software-DGE (gpsimd) DMAs; X^T/attnt/qt/kt
    are written bf16 by their producing DVE op for free.
  - 1/denominator runs as exp(-ln(denom)) on the ACT engine. DVE's exact
    reciprocal is iterative divide (~6 cyc/elem on HW; the CoreSim cost
    model under-times it ~5x) and cost ~200us/rep in normalize chains.
    Exp and Ln share one activation table only if the table-choice pass is
    steered to the combined set (_pin_combined_exp_ln_table), else it
    reloads tables between every pair at 1.3us each.
  - The custom-DVE reciprocal_approx_fast op miscompiles on this
    toolchain's HW path (sim executes its numpy reference - correct in
    sim, garbage on HW). Avoid custom DVE ops here.
  - Projection chains run so-outer/ch-inner with both w_proj chunks
    preloaded, so every chain on q<512 issues before the last qch=1
    normalize; the post-attention serial tail is ~halved (-58us/rep).
  - Bias broadcasts run on the idle gpsimd engine (partition_broadcast),
    not as PE ones-matmuls.
Even/odd head pairs are emitted back-to-back so their K=64 score matmuls
overlap in disjoint PE row groups (tile_position (0,0)/(64,0), measured 4x).
End-to-end error vs the fp32 reference: ~5.4e-3 (budget 2e-2).
"""

from contextlib import ExitStack

import numpy as np

import concourse.mybir as mybir
import concourse.tile as tile
from concourse import bacc
from concourse.bass_utils import run_bass_kernel_spmd
from concourse.hw_specs import get_activation_tables
from concourse.masks import make_identity


def _pin_combined_exp_ln_table(arch):
    """Make Exp and Ln resolve to the one table containing both.

    The act-table insertion pass assigns each activation the FIRST table
    containing its function: Exp -> exp_and_others, Ln -> natural_log.
    A kernel using both then reloads tables between every pair (1.3us per
    load). Shrinking the cached per-table function sets so Exp/Ln appear
    only in the combined natural_log_exp table (true act_func_set_id is
    positional and the order is untouched) yields a single table load.
    """
    tabs = get_activation_tables(arch)
    exp = mybir.ActivationFunctionType.Exp
    ln = mybir.ActivationFunctionType.Ln
    combined = None
    for name, s in tabs.items():
        if exp in s and ln in s:
            combined = name
            break
    if combined is None:
        return
    for name, s in tabs.items():
        if name != combined:
            s.discard(exp)
            s.discard(ln)

S = 1024  # sequence length (per core batch element)
D = 1024  # embed dim
H = 16  # heads
HD = 64  # head dim
P = 128  # partitions
NCORES = 8
NG = 4  # head groups (4 heads / 256 channels each)
GC = 256  # channels per group
SCALE = 1.0 / 8.0  # 1/sqrt(HD)

F32 = mybir.dt.float32
F32R = mybir.dt.float32r
BF16 = mybir.dt.bfloat16
AF = mybir.ActivationFunctionType


def make_pools(ctx, tc):
    return {
        "const": ctx.enter_context(tc.tile_pool(name="const", bufs=1)),
        "xtp": ctx.enter_context(tc.tile_pool(name="xtp", bufs=1)),
        "xinp": ctx.enter_context(tc.tile_pool(name="xinp", bufs=3)),
        "wblkp": ctx.enter_context(tc.tile_pool(name="wblkp", bufs=4)),
        "qkp": ctx.enter_context(tc.tile_pool(name="qkp", bufs=2)),
        "vgp": ctx.enter_context(tc.tile_pool(name="vgp", bufs=2)),
        "ptp": ctx.enter_context(tc.tile_pool(name="ptp", bufs=2)),
        "wpp": ctx.enter_context(tc.tile_pool(name="wpp", bufs=1)),
        "smp": ctx.enter_context(tc.tile_pool(name="smp", bufs=4)),
        "ps": ctx.enter_context(tc.tile_pool(name="ps", bufs=2, space="PSUM")),
    }


def emit_mha(
    pools, tc, out, x, wqkv, bqkv, wproj, bproj,
    two_ko=True, gp_bcast=True, pt_bf16=True, act_recip=True, bf16_ops=True,
    xin_bf16=False, wdma_2ko=False, so_outer=True, dma_xpose=False,
    psum_out=False,
):
    nc = tc.nc

    const = pools["const"]
    xt_pool = pools["xtp"]
    xin_pool = pools["xinp"]
    wblk_pool = pools["wblkp"]
    qk_pool = pools["qkp"]
    vg_pool = pools["vgp"]
    sm_pool = pools["smp"]
    ps = pools["ps"]

    # ---- start the big input DMAs first (X tiles; W streams follow via
    # the group loop) so the DMA queues ramp while constants are built.
    # With bf16_ops, X is cast fp32->bf16 in-flight (software-DGE DMA) so
    # the transposes run at 1 cycle/row with 2-byte stationary loads. ----
    xi_dt = BF16 if (xin_bf16 or dma_xpose) else F32R
    xi_dma = (
        nc.gpsimd.dma_start if (xin_bf16 or dma_xpose) else nc.sync.dma_start
    )
    xins = []
    for so in range(8):
        xin = xin_pool.tile([P, D], xi_dt, tag="xin", bufs=3, name="xin")
        if so == 0:
            # halve the first tile's DMA so the first transposes start
            # ~1.5us sooner; later tiles overlap compute anyway
            xi_dma(xin[:, 0:512], x[0:P, 0:512])
            xi_dma(xin[:, 512:D], x[0:P, 512:D])
        else:
            xi_dma(xin, x[so * P : (so + 1) * P, :])
        xins.append(xin)

    # per-ko DMA split: consumers start when their 128-row slice lands,
    # instead of waiting for the whole 1MB block transfer. With bf16_ops
    # the weights are cast fp32->bf16 in-flight by the software-DGE
    # (gpsimd) DMA, halving every stationary weight load into the PE.
    def emit_w_dma(g):
        w_dt = BF16 if bf16_ops else F32R
        w_dma = nc.gpsimd.dma_start if bf16_ops else nc.sync.dma_start
        wq = wblk_pool.tile([P, 8, GC], w_dt, tag="wblk", name="wq")
        wk = wblk_pool.tile([P, 8, GC], w_dt, tag="wblk", name="wk")
        wv = wblk_pool.tile([P, 8, GC], w_dt, tag="wblk", name="wv")
        kstep = 2 if wdma_2ko else 1
        for ko in range(0, 8, kstep):
            rows = slice(ko * P, (ko + kstep) * P)
            ksl = slice(ko, ko + kstep)
            cg = slice(g * GC, (g + 1) * GC)
            w_dma(wq[:, ksl], wqkv[rows, cg].rearrange("(k p) c -> p k c", p=P))
            w_dma(
                wk[:, ksl],
                wqkv[rows, D + g * GC : D + (g + 1) * GC].rearrange(
                    "(k p) c -> p k c", p=P
                ),
            )
            w_dma(
                wv[:, ksl],
                wqkv[rows, 2 * D + g * GC : 2 * D + (g + 1) * GC].rearrange(
                    "(k p) c -> p k c", p=P
                ),
            )
        return wq, wk, wv


    # ---- constants / biases ----
    # f32r tiles cannot be memset directly (ISA restriction); build f32
    # versions and DVE-copy, which performs the f32 -> f32r rounding.
    identf = const.tile([P, P], F32, name="identf")
    make_identity(nc, identf)
    ident = const.tile([P, P], xi_dt, name="ident")
    nc.vector.tensor_copy(ident, identf)
    onesf = const.tile([P, P], F32, name="onesf")
    nc.vector.memset(onesf, 1.0)
    ones64 = const.tile([1, HD], F32R, name="ones64")
    nc.vector.tensor_copy(ones64, onesf[0:1, 0:HD])

    # b_qkv striped per-partition: b_sb[p, col] = b_qkv[col*128 + p]
    b_sb = const.tile([P, 24], F32, name="b_sb")
    nc.sync.dma_start(b_sb, bqkv.rearrange("(col p) -> p col", p=P))
    bq_s = const.tile([P, 8], F32, name="bq_s")  # pre-scaled Q bias
    nc.vector.tensor_scalar_mul(bq_s, b_sb[:, 0:8], SCALE)

    # V and proj biases broadcast to [128, D] on the (idle) gpsimd engine
    bvrow = xin_pool.tile([1, D], F32, tag="xin", name="bvrow")
    nc.gpsimd.dma_start(bvrow, bqkv[2 * D : 3 * D].rearrange("(a c) -> a c", a=1))
    bvb = const.tile([P, D], F32, name="bvb")
    nc.gpsimd.partition_broadcast(bvb, bvrow)

    # ---- X^T, split into two half-sequence tiles so early QKV matmuls
    # depend only on the first 32 transposes, not all 64 ----
    xt_dt = BF16 if bf16_ops else F32R
    xth = [
        xt_pool.tile([P, 8, S // 2], xt_dt, tag=f"xt{h}", name=f"xt{h}")
        for h in range(2)
    ]
    for so in range(8):
        xin = xins[so]
        for do in range(8):
            if dma_xpose:
                # XBAR transpose-DMA (2-byte dtypes, SBUF->SBUF): no PE
                # transpose instruction, no PSUM round-trip, no DVE copy
                nc.sync.dma_start_transpose(
                    xth[so // 4][:, do, (so % 4) * P : (so % 4 + 1) * P],
                    xin[:, do * P : (do + 1) * P],
                )
            else:
                pst = ps.tile([P, P], xi_dt, tag="sc", bufs=2, name="pst")
                nc.tensor.transpose(pst, xin[:, do * P : (do + 1) * P], ident)
                nc.vector.tensor_copy(
                    xth[so // 4][:, do, (so % 4) * P : (so % 4 + 1) * P], pst
                )

    def xt_slice(ko, s0, s1):
        # contiguous [s0:s1) slice of X^T row-block ko; must stay in one half
        h = s0 // 512
        assert (s1 - 1) // 512 == h
        return xth[h][:, ko, s0 - h * 512 : s1 - h * 512]

    attnt = xt_pool.tile([P, 8, S], BF16 if bf16_ops else F32R, tag="attnt", name="attnt")

    # ---- per head-group: QKV projection then attention ----
    pv_dt = BF16 if pt_bf16 else F32R
    for g in range(4):
        wq, wk, wv = emit_w_dma(g)
        qk_dt = BF16 if bf16_ops else F32R
        qt = qk_pool.tile([P, 2, S], qk_dt, tag="qt", name="qt")
        kt = qk_pool.tile([P, 2, S], qk_dt, tag="kt", name="kt")
        for cb in range(2):
            for qch in range(2):
                sl = slice(qch * 512, (qch + 1) * 512)
                psq = ps.tile([P, 512], F32, tag="mm", bufs=2, name="psq")
                for ko in range(8):
                    nc.tensor.matmul(
                        psq,
                        lhsT=wq[:, ko, cb * P : (cb + 1) * P],
                        rhs=xt_slice(ko, qch * 512, (qch + 1) * 512),
                        start=(ko == 0),
                        stop=(ko == 7),
                    )
                nc.vector.tensor_scalar(
                    qt[:, cb, sl], psq,
                    SCALE, bq_s[:, 2 * g + cb : 2 * g + cb + 1],
                    mybir.AluOpType.mult, mybir.AluOpType.add,
                )
                psk = ps.tile([P, 512], F32, tag="mm", bufs=2, name="psk")
                for ko in range(8):
                    nc.tensor.matmul(
                        psk,
                        lhsT=wk[:, ko, cb * P : (cb + 1) * P],
                        rhs=xt_slice(ko, qch * 512, (qch + 1) * 512),
                        start=(ko == 0),
                        stop=(ko == 7),
                    )
                nc.vector.tensor_scalar(
                    kt[:, cb, sl], psk,
                    b_sb[:, 8 + 2 * g + cb : 8 + 2 * g + cb + 1], None,
                    mybir.AluOpType.add,
                )

        # V for this group: [s, 4 heads x (64 + ones col)]
        vg = vg_pool.tile([P, 8, 4, HD + 1], pv_dt, tag="vg", name="vg")
        nc.vector.tensor_copy(
            vg[:, :, :, HD], onesf[:, 0:32].rearrange("p (a b) -> p a b", a=8)
        )
        for so in range(8):
            psv = ps.tile([P, GC], F32, tag="mm", bufs=2, name="psv")
            for ko in range(8):
                nc.tensor.matmul(
                    psv,
                    lhsT=xt_slice(ko, so * P, (so + 1) * P),
                    rhs=wv[:, ko, :],
                    start=(ko == 0),
                    stop=(ko == 7),
                )
            nc.vector.tensor_add(
                out=vg[:, so, :, 0:HD],
                in0=psv.rearrange("p (h c) -> p h c", h=4),
                in1=bvb[:, g * GC : (g + 1) * GC].rearrange("p (h c) -> p h c", h=4),
            )

        _attention_group(
            nc, pools, g, 0, qt, kt, vg, attnt,
            two_ko=two_ko, gp_bcast=gp_bcast, pt_bf16=pt_bf16,
            act_recip=act_recip, ones64=ones64,
        )

    _emit_proj(
        pools, nc, out, wproj, bproj, attnt, bf16_ops=bf16_ops,
        so_outer=so_outer, psum_out=psum_out,
    )


def _attention_group(
    nc, pools, g, gi2, qt, kt, vg, attnt,
    two_ko, gp_bcast, pt_bf16, act_recip, ones64,
):
    pt_pool = pools["ptp"]
    sm_pool = pools["smp"]
    ps = pools["ps"]
    pv_dt = BF16 if pt_bf16 else F32R
    if True:
        for pp in range(2):
            heads = (2 * pp, 2 * pp + 1)  # even, odd within group
            for qch in range(2):
                qsl = slice(qch * 512, (qch + 1) * 512)
                # two separate accumulators (not one [P,2,512] tile): a
                # shared tile serializes the pair's drains behind a joint
                # ln/exp and delays the next iteration's chains (+6us)
                pvs = [
                    ps.tile([P, 512], F32, tag="pv", bufs=2, name=f"pspv{i}")
                    for i in range(2)
                ]
                kw = 2 if two_ko else 1  # exp width in ko tiles
                for kp in range(8 // kw):
                    scs, pts = [], []
                    for i, hb in enumerate(heads):
                        scs.append(
                            ps.tile([P, kw, 512], F32, tag="sc", bufs=2, name="pssc")
                        )
                        pts.append(
                            pt_pool.tile(
                                [P, kw, 512], pv_dt, tag="pt",
                                bufs=4 if pt_bf16 else 2, name="pt",
                            )
                        )
                    for j in range(kw):
                        ko = kw * kp + j
                        for i, hb in enumerate(heads):
                            poff = (hb % 2) * HD
                            nc.tensor.matmul(
                                scs[i][:, j],
                                lhsT=kt[poff : poff + HD, pp, ko * P : (ko + 1) * P],
                                rhs=qt[poff : poff + HD, pp, qsl],
                                start=True,
                                stop=True,
                            )
                    for i in range(2):
                        nc.scalar.activation(pts[i], scs[i], AF.Exp)
                    for j in range(kw):
                        ko = kw * kp + j
                        for i, hb in enumerate(heads):
                            nc.tensor.matmul(
                                pvs[i][0 : HD + 1],
                                lhsT=vg[:, ko, 4 * gi2 + hb],
                                rhs=pts[i][:, j],
                                start=(ko == 0),
                                stop=(ko == 7),
                            )
                for i, hb in enumerate(heads):
                    poff = (hb % 2) * HD
                    rec_dt = F32 if gp_bcast else F32R
                    rec = sm_pool.tile([1, 512], rec_dt, tag="rec", bufs=2, name="rec")
                    if act_recip:
                        # 1/denom as exp(-ln(denom)) on ACT: denom is a sum of
                        # exps (positive, no edge cases), and ln+exp share one
                        # activation table. DVE's exact reciprocal is iterative
                        # divide at ~6 cyc/elem on HW - 3.2us per call here.
                        lnd = sm_pool.tile([1, 512], F32, tag="rec", bufs=2, name="lnd")
                        nc.scalar.activation(lnd, pvs[i][HD : HD + 1, :], AF.Ln)
                        nc.scalar.activation(rec, lnd, AF.Exp, scale=-1.0)
                    else:
                        nc.vector.reciprocal(rec, pvs[i][HD : HD + 1, :])
                    recb = sm_pool.tile([HD, 512], F32, tag="recb", bufs=2, name="recb")
                    if gp_bcast:
                        nc.gpsimd.partition_broadcast(recb, rec)
                    else:
                        psbc = ps.tile([HD, 512], F32, tag="bc", bufs=1, name="psbc")
                        nc.tensor.matmul(
                            psbc, lhsT=ones64, rhs=rec, start=True, stop=True
                        )
                        nc.vector.tensor_copy(recb, psbc)
                    nc.vector.tensor_mul(
                        out=attnt[poff : poff + HD, 2 * g + pp, qsl],
                        in0=pvs[i][0:HD, :],
                        in1=recb,
                    )


def _emit_proj(
    pools, nc, out, wproj, bproj, attnt,
    bf16_ops=True, so_outer=True, psum_out=False,
):
    xin_pool = pools["xinp"]
    const = pools["const"]
    wp_pool = pools["wpp"]
    sm_pool = pools["smp"]
    ps = pools["ps"]

    # proj bias, deferred: not needed until now
    if psum_out:
        # bias lands as a rank-1 (K=1) matmul closing each chain: ones
        # column (x) b_proj row, all bf16 - no DVE add, no SBUF hop, and
        # the out-DMA reads PSUM directly
        bprow = xin_pool.tile([1, D], BF16, tag="xin", name="bprow")
        nc.gpsimd.dma_start(bprow, bproj.rearrange("(a c) -> a c", a=1))
        ones1b = const.tile([1, P], BF16, name="ones1b")
        nc.vector.memset(ones1b, 1.0)
    else:
        bprow = xin_pool.tile([1, D], F32, tag="xin", name="bprow")
        nc.gpsimd.dma_start(bprow, bproj.rearrange("(a c) -> a c", a=1))
        bpb = const.tile([P, D], F32, name="bpb")
        nc.gpsimd.partition_broadcast(bpb, bprow)

    # ---- output projection. Both 512-column w_proj chunks are preloaded,
    # and chains run so-outer / ch-inner: a chain (so, ch) depends only on
    # attnt columns of its own q-range, so all so<4 chains (q < 512, ready
    # after the qch=0 normalizes) issue before the final qch=1 normalize
    # gates the so>=4 tail - shortest possible post-attention serial tail.
    wps = []
    for ch in range(2):
        sl = slice(ch * 512, (ch + 1) * 512)
        if bf16_ops:
            # stationary operands load faster at 2 bytes; convert on the
            # (idle) gpsimd engine as each 2-ko slice lands
            wpf = wp_pool.tile([P, 8, 512], F32R, tag="wpf", name="wpf")
            wp = wp_pool.tile([P, 8, 512], BF16, tag="wp", bufs=2, name="wp")
            for ko in range(8):
                nc.sync.dma_start(wpf[:, ko], wproj[ko * P : (ko + 1) * P, sl])
            for k2 in range(4):
                nc.gpsimd.tensor_copy(
                    wp[:, 2 * k2 : 2 * k2 + 2], wpf[:, 2 * k2 : 2 * k2 + 2]
                )
        else:
            wp = wp_pool.tile([P, 8, 512], F32R, tag="wp", bufs=2, name="wp")
            for ko in range(8):
                nc.sync.dma_start(wp[:, ko], wproj[ko * P : (ko + 1) * P, sl])
        wps.append(wp)
    order = (
        [(so, ch) for so in range(8) for ch in range(2)]
        if so_outer
        else [(so, ch) for ch in range(2) for so in range(8)]
    )
    if True:
        for so, ch in order:
            sl = slice(ch * 512, (ch + 1) * 512)
            # alternate PSUM tags: the attention-phase "sc" slots are free
            # during proj, doubling chains in flight at the kernel tail
            if (2 * so + ch) % 2 == 0:
                psp = ps.tile([P, 512], F32, tag="mm", bufs=2, name="psp")
            else:
                psp = ps.tile([P, 2, 512], F32, tag="sc", bufs=2, name="pspw")[:, 0]
            for ko in range(8):
                nc.tensor.matmul(
                    psp,
                    lhsT=attnt[:, ko, so * P : (so + 1) * P],
                    rhs=wps[ch][:, ko, :],
                    start=(ko == 0),
                    stop=(ko == 7) and not psum_out,
                )
            if psum_out:
                nc.tensor.matmul(
                    psp, lhsT=ones1b, rhs=bprow[:, sl],
                    start=False, stop=True,
                )
                # drain via the ACT engine (idle during proj); DVE stays free
                ot = sm_pool.tile([P, 512], F32, tag="ot", bufs=3, name="ot")
                nc.scalar.copy(ot, psp)
                nc.sync.dma_start(out[so * P : (so + 1) * P, sl], ot)
            else:
                ot = sm_pool.tile([P, 512], F32, tag="ot", bufs=3, name="ot")
                nc.vector.tensor_add(out=ot, in0=psp, in1=bpb[:, sl])
                nc.sync.dma_start(out[so * P : (so + 1) * P, sl], ot)


def build_nc(
    repeat=1, two_ko=True, gp_bcast=True, pt_bf16=True, act_recip=True,
    bf16_ops=True, xin_bf16=False, wdma_2ko=False, so_outer=True,
    dma_xpose=False, psum_out=False,
):
    nc = bacc.Bacc("TRN2", target_bir_lowering=False, debug=False, num_devices=NCORES)
    if act_recip:
        _pin_combined_exp_ln_table(nc.m.arch)
    x = nc.dram_tensor("query", [S, D], F32R, kind="ExternalInput").ap()
    wqkv = nc.dram_tensor("w_qkv", [D, 3 * D], F32R, kind="ExternalInput").ap()
    bqkv = nc.dram_tensor("b_qkv", [3 * D], F32, kind="ExternalInput").ap()
    wproj = nc.dram_tensor("w_proj", [D, D], F32R, kind="ExternalInput").ap()
    bproj = nc.dram_tensor("b_proj", [D], F32, kind="ExternalInput").ap()
    out = nc.dram_tensor("out", [S, D], F32, kind="ExternalOutput").ap()
    with (
        tile.TileContext(nc) as tc,
        ExitStack() as ctx,
        nc.allow_low_precision(reason="float32r matmul pipeline (~1e-3)"),
    ):
        pools = make_pools(ctx, tc)
        for _ in range(repeat):
            emit_mha(
                pools, tc, out, x, wqkv, bqkv, wproj, bproj,
                two_ko=two_ko, gp_bcast=gp_bcast, pt_bf16=pt_bf16,
                act_recip=act_recip, bf16_ops=bf16_ops, xin_bf16=xin_bf16,
                wdma_2ko=wdma_2ko, so_outer=so_outer, dma_xpose=dma_xpose,
                psum_out=psum_out,
            )
    nc.compile()
    return nc


_NC_CACHE = None


def _get_nc():
    global _NC_CACHE
    if _NC_CACHE is None:
        _NC_CACHE = build_nc()
    return _NC_CACHE


def make_in_maps(query, w_qkv, b_qkv, w_proj, b_proj):
    f = np.float32
    shared = {
        "w_qkv": np.ascontiguousarray(w_qkv, dtype=f),
        "b_qkv": np.ascontiguousarray(b_qkv, dtype=f),
        "w_proj": np.ascontiguousarray(w_proj, dtype=f),
        "b_proj": np.ascontiguousarray(b_proj, dtype=f),
    }
    return [
        {"query": np.ascontiguousarray(query[i], dtype=f), **shared}
        for i in range(NCORES)
    ]


def kernel(query, w_qkv, b_qkv, w_proj, b_proj):
    nc = _get_nc()
    in_maps = make_in_maps(query, w_qkv, b_qkv, w_proj, b_proj)
    res = run_bass_kernel_spmd(nc, in_maps, core_ids=list(range(NCORES)))
    return np.stack([res.results[i]["out"] for i in range(NCORES)]).astype(np.float32)

